# revision 49
# baseline (speedup 1.0000x reference)
"""CaptioningRNN (LSTM + spatial attention + vocab loss) on 8 Trainium2 cores.

Strategy:
 - The three big parallel matmul groups (CNN-feature projection, x@Wx
   precompute, vocab logits + logsumexp) are sharded 8 ways.
 - The sequential LSTM is sharded over the 4H gate dimension: core c owns a
   128-wide slice of each gate (aligned with hidden slice c). Each step ends
   with a fused AllGather carrying (a) the core's transposed h-slice (feeds
   the next step's matmuls on every core) and (b) partial attention scores
   for the core's hidden slice (summed on every core after the gather).
 - Attention scores are computed batch-major directly from h_sl via a
   broadcast-multiply + reduce against A_batch[n, p, h]; attn @ Wattn is
   eliminated via the B[n, sc, p] = A[n, :, p] @ Wattn precompute and a
   single broadcast-multiply + reduce over p, with the softmax
   normalization (1/z) folded into the final fused scalar_tensor_tensor.
 - Loss: logsumexp partials per vocab shard (no max subtraction - logits are
   O(3)), label logits via host-gathered W_vocab columns, one final AllGather,
   then a replicated tiny reduction. b_vocab[y] mask term added on host.

build(T, reps=K) emits the whole computation K times inside one NEFF
(weights preloaded once, all work tiles shared across reps via tags) so the
per-execution HW time can be measured as the wall-clock slope over K,
amortizing out the fixed axon dispatch overhead.
"""
import sys, os, time

sys.path.insert(0, "/opt/trn_rl_repo")

import numpy as np
import ml_dtypes

import concourse.bass as bass
import concourse.bacc as bacc
import concourse.mybir as mybir
import concourse.tile as tile
import concourse.bass_isa as bass_isa
from concourse.bass_utils import run_bass_kernel_spmd


BF16 = ml_dtypes.bfloat16
F32 = mybir.dt.float32
BF = mybir.dt.bfloat16
U8 = mybir.dt.uint8

NCORES = 8
N = 128          # batch
TCAP = 31
CIN = 1280
WD = 512         # embed dim
H = 1024         # hidden
V = 10000        # vocab
P = 16           # spatial positions
HS = H // NCORES      # 128 hidden slice
SC = 4 * HS           # 512 a-columns per core
VS = V // NCORES      # 1250 vocab slice
AluOp = mybir.AluOpType
Act = mybir.ActivationFunctionType

HT_BYTES = 128 * 128 * 2           # h^T slice, bf16
SC_BYTES = 128 * 16 * 4            # partial scores [n, p], fp32
PAY = HT_BYTES + SC_BYTES          # per-step AG payload bytes


def build(T, reps=1, abl=()):
    nc = bacc.Bacc("TRN2", target_bir_lowering=False, debug=False,
                   num_devices=NCORES)
    NT = T * N

    def din(name, shape, dt):
        return nc.dram_tensor(name, shape, dt, kind="ExternalInput").ap()

    imgsT = din("imgsT", [CIN, P * N], BF)      # (cin, p, n) layout
    # xembT2: [128, (kt, t, n)] so one DMA loads a step's x_t k-tiles
    xembT2 = din("xembT2", [128, 4 * T * 128], BF)
    wprojS = din("wprojS", [CIN, HS], BF)
    bprojS = din("bprojS", [HS, 1], F32)
    WxS = din("WxS", [WD, SC], BF)
    brow = din("brow", [1, SC], BF)
    WhS = din("WhS", [H, SC], BF)
    WattnS = din("WattnS", [H, SC], BF)
    wvoc = din("wvoc", [H, VS], BF)
    bvoc = din("bvoc", [1, VS], BF)
    wyT = din("wyT", [T, 128, H], BF)
    mask_f = din("mask_f", [N, T], F32)
    ident = din("ident", [128, 128], BF)
    ones1 = din("ones1", [128, 1], BF)
    onesrow = din("onesrow", [1, 128], BF)

    loss_out = nc.dram_tensor("loss", [1, 1], F32, kind="ExternalOutput").ap()

    VCH = [(0, 512), (512, 512), (1024, VS - 1024)]  # vocab chunks
    ZB = N * T * 4
    rg = [list(range(NCORES))]

    with tile.TileContext(nc) as tc:
        with (
            tc.tile_pool(name="dram", bufs=1, space="DRAM") as dram,
            tc.tile_pool(name="persist", bufs=1) as pp,
            tc.tile_pool(name="work", bufs=3) as wp,
            tc.tile_pool(name="psB", bufs=1, space="PSUM") as psB,
        ):
            # ---------- persistent SBUF: weights, loaded once ----------
            whs_sb = pp.tile([128, 8, SC], BF, name="whs_sb")
            wvoc_sb = pp.tile([128, 8, VS], BF, name="wvoc_sb")
            bvoc_sb = pp.tile([1, VS], BF, name="bvoc_sb")
            ident_sb = pp.tile([128, 128], BF, name="ident_sb")
            ones1_sb = pp.tile([128, 1], BF, name="ones1_sb")
            onesrow_sb = pp.tile([1, 128], BF, name="onesrow_sb")
            bproj_sb = pp.tile([HS, 1], F32, name="bproj_sb")
            mask_sb = pp.tile([N, T], F32, name="mask_sb")
            wproj_sb = pp.tile([128, 10, 128], BF, name="wproj_sb")
            watt_sb = pp.tile([128, 8, SC], BF, name="watt_sb")
            wxs_sb = pp.tile([128, 4, SC], BF, name="wxs_sb")
            brow_sb = pp.tile([1, SC], BF, name="brow_sb")

            for dst, src in [(ident_sb, ident), (ones1_sb, ones1),
                             (onesrow_sb, onesrow),
                             (bproj_sb, bprojS),
                             (mask_sb, mask_f), (bvoc_sb, bvoc),
                             (brow_sb, brow)]:
                nc.sync.dma_start(dst[:], src[:])
            for kt in range(8):
                nc.sync.dma_start(whs_sb[:, kt, :], WhS[bass.ts(kt, 128), :])
                nc.sync.dma_start(wvoc_sb[:, kt, :], wvoc[bass.ts(kt, 128), :])
                nc.sync.dma_start(watt_sb[:, kt, :], WattnS[bass.ts(kt, 128), :])
            for kt in range(10):
                nc.sync.dma_start(wproj_sb[:, kt, :], wprojS[bass.ts(kt, 128), :])
            for kt in range(4):
                nc.sync.dma_start(wxs_sb[:, kt, :], WxS[bass.ts(kt, 128), :])

            for rep in range(reps):
                emit(nc, tc, dram, wp, psB, rep, T,
                     dict(imgsT=imgsT, xembT2=xembT2, wyT=wyT,
                          loss_out=loss_out,
                          whs_sb=whs_sb, wvoc_sb=wvoc_sb, bvoc_sb=bvoc_sb,
                          ident_sb=ident_sb,
                          ones1_sb=ones1_sb, onesrow_sb=onesrow_sb,
                          bproj_sb=bproj_sb, mask_sb=mask_sb,
                          wproj_sb=wproj_sb, watt_sb=watt_sb,
                          wxs_sb=wxs_sb, brow_sb=brow_sb),
                     VCH, ZB, rg, abl)

    nc.compile()
    return nc


def emit(nc, tc, dram, wp, psB, rep, T, g, VCH, ZB, rg, abl=()):
    """One full forward pass; work tiles share buffers across reps via tags.

    abl: ablation flags for timing experiments only (breaks correctness):
      "novocab" - skip per-row vocab logsumexp work
      "noattn"  - skip score/softmax/attention path
      "noag"    - replace the per-step AllGather with local copies
    """
    R = f"r{rep}"
    imgsT, xembT2, wyT = g["imgsT"], g["xembT2"], g["wyT"]
    loss_out = g["loss_out"]
    whs_sb, wvoc_sb, bvoc_sb = g["whs_sb"], g["wvoc_sb"], g["bvoc_sb"]
    ident_sb = g["ident_sb"]
    ones1_sb, onesrow_sb = g["ones1_sb"], g["onesrow_sb"]
    bproj_sb, mask_sb = g["bproj_sb"], g["mask_sb"]
    wproj_sb, watt_sb = g["wproj_sb"], g["watt_sb"]
    wxs_sb, brow_sb = g["wxs_sb"], g["brow_sb"]

    # per-rep state tiles (shared buffers across reps via tags)
    ident16 = wp.tile([128, P, 128], BF, name=f"{R}ident16", tag="ident16",
                      bufs=1)
    for p in range(P):
        nc.scalar.copy(ident16[:, p, :], g["ident_sb"][:])
    atm = wp.tile([128, P, N], BF, name=f"{R}atm", tag="atm", bufs=1)
    ab = wp.tile([128, P, HS], BF, name=f"{R}ab", tag="ab", bufs=1)  # A[n,p,h]
    at_all = wp.tile([128, 8, P, N], BF, name=f"{R}at_all", tag="at_all", bufs=1)
    b_sb = wp.tile([128, P, SC], BF, name=f"{R}b_sb", tag="b_sb", bufs=1)
    zc_sb = wp.tile([128, T, 3], F32, name=f"{R}zc_sb", tag="zc_sb", bufs=1)
    c_sb = wp.tile([128, 128], F32, name=f"{R}c_sb", tag="c_sb", bufs=1)

    # ---------- P1: projection  A^T[my hslice, (p,n)] ----------
    ps_a = psB.tile([128, P * N], F32, name=f"{R}ps_a", tag="ps_big")
    for kt in range(10):
        imgs_kt = wp.tile([128, P * N], BF, name=f"{R}imgs{kt}",
                          tag="imgs_kt", bufs=2)
        nc.sync.dma_start(imgs_kt[:], imgsT[bass.ts(kt, 128), :])
        for ch in range(4):
            nc.tensor.matmul(ps_a[:, bass.ts(ch, 512)],
                             wproj_sb[:, kt, :],
                             imgs_kt[:, bass.ts(ch, 512)],
                             start=(kt == 0), stop=(kt == 9))
    # + b_proj (per-partition), cast bf16
    nc.scalar.activation(atm[:].rearrange("p n q -> p (n q)"),
                         ps_a[:], Act.Identity, bias=bproj_sb[:])
    # A batch-major copy: ab[n, p, h] via 16 PE transposes of atm[:, p, :]
    for p in range(P):
        ps_t2 = psB.tile([128, 128], BF, name=f"{R}ps_ab{p}", tag="ps_tr",
                         bufs=1)
        nc.tensor.transpose(ps_t2[:], atm[:, p, :], ident_sb[:])
        nc.scalar.copy(ab[:, p, :], ps_t2[:])
    # h0^T my slice = mean over positions (strided view [h, n, p])
    h0t_f = wp.tile([128, 128], F32, name=f"{R}h0t_f", tag="h0t_f", bufs=1)
    nc.vector.tensor_reduce(h0t_f[:], atm[:].transpose([0, 2, 1]),
                            mybir.AxisListType.X, AluOp.add)
    nc.scalar.mul(h0t_f[:], h0t_f[:], 1.0 / P)
    h0t_b = wp.tile([128, 128], BF, name=f"{R}h0t_b", tag="h0t_b", bufs=1)
    nc.scalar.copy(h0t_b[:], h0t_f[:])
    # c0 = h0 (batch-major my hidden slice)
    ps_tr0 = psB.tile([128, 128], BF, name=f"{R}ps_tr", tag="ps_tr",
                      bufs=1)
    nc.tensor.transpose(ps_tr0[:], h0t_b[:], ident_sb[:])
    nc.scalar.copy(c_sb[:], ps_tr0[:])
    c_bf = wp.tile([128, 128], BF, name=f"{R}c_bf", tag="c_bf", bufs=1)
    nc.scalar.copy(c_bf[:], ps_tr0[:])

    # scores partial [n, p] = sum_h h[n,h] * A[n,p,h]  (scale 1/32 folded
    # into the post-AG exp), packed with h^T into the per-step AllGather.
    def scores_and_ag(step, hT_ap, hb_ap):
        pay = dram.tile([PAY], U8, name=f"{R}pay{step}", tag="pay", bufs=3)
        if "noattn" not in abl:
            e_b = wp.tile([128, P, HS], BF, name=f"{R}e_{step}", tag="e_b",
                          bufs=2)
            nc.vector.tensor_mul(
                e_b[:], ab[:],
                hb_ap.unsqueeze(1).broadcast_to([128, P, HS]))
            sc_part = wp.tile([128, P], F32, name=f"{R}scp{step}",
                              tag="sc_part", bufs=2)
            nc.vector.tensor_reduce(sc_part[:], e_b[:],
                                    mybir.AxisListType.X, AluOp.add)
            nc.sync.dma_start(
                pay[HT_BYTES:PAY].bitcast(F32)
                .rearrange("(n q) -> n q", n=128),
                sc_part[:])
        nc.sync.dma_start(
            pay[0:HT_BYTES].rearrange("(p b) -> p b", p=128),
            hT_ap.bitcast(mybir.dt.uint8))
        gat = dram.tile([NCORES, PAY], U8, name=f"{R}gat{step}", tag="gat",
                        bufs=3, addr_space="Shared")
        if "noag" in abl:
            nc.sync.dma_start(gat[0], pay[:])
        else:
            nc.gpsimd.collective_compute(
                "AllGather", AluOp.bypass, replica_groups=rg,
                ins=[pay.opt()], outs=[gat.opt()])
        return gat

    gat = scores_and_ag(0, h0t_b[:], c_bf[:])

    # ---------- at_all: gather A^T from all cores ----------
    pay_a = dram.tile([128, P * N], BF, name=f"{R}pay_a", tag="pay_a")
    nc.sync.dma_start(pay_a[:], atm[:].rearrange("p n q -> p (n q)"))
    gat_a = dram.tile([NCORES * 128, P * N], BF, name=f"{R}gat_a",
                      tag="gat_a", addr_space="Shared")
    nc.gpsimd.collective_compute(
        "AllGather", AluOp.bypass, replica_groups=rg,
        ins=[pay_a.opt()], outs=[gat_a.opt()])
    gav = gat_a[:].rearrange("(c p) f -> c p f", c=NCORES)
    for kt in range(8):
        nc.sync.dma_start(
            at_all[:, kt, :, :].rearrange("p n q -> p (n q)"), gav[kt])

    # ---------- B precompute: b_sb[n, p, :] = A[n, :, p] @ WattnS ----------
    for p in range(P):
        ps_b = psB.tile([128, SC], F32, name=f"{R}ps_b{p}", tag="ps_mm",
                        bufs=3)
        for kt in range(8):
            nc.tensor.matmul(ps_b[:], at_all[:, kt, p, :],
                             watt_sb[:, kt, :],
                             start=(kt == 0), stop=(kt == 7))
        nc.scalar.copy(b_sb[:, p, :], ps_b[:])

    # ---------- recurrence ----------
    xv = xembT2[:].rearrange("p (k t n) -> p k t n", k=4, t=T)
    pay2 = dram.tile([N * T * 4 + T * 128 * 4], U8, name=f"{R}pay2", tag="pay2")

    def vocab_prefetch(trow):
        wyt_sb = wp.tile([128, H], BF, name=f"{R}wyt{trow}", tag="wyt_sb",
                         bufs=2)
        nc.sync.dma_start(wyt_sb[:], wyT[trow])
        return wyt_sb

    def vocab_row(trow, hT_sb, wyt_sb):
        # logsumexp partials + label logits for hs row `trow` (h_{trow+1})
        for ci, (off, ln) in enumerate(VCH):
            ps_v = psB.tile([128, 512], F32, name=f"{R}ps_v{trow}_{ci}",
                            tag="ps_mm", bufs=3)
            for kt in range(8):
                nc.tensor.matmul(ps_v[:, :ln], hT_sb[:, kt, :],
                                 wvoc_sb[:, kt, off:off + ln],
                                 start=(kt == 0), stop=False)
            nc.tensor.matmul(ps_v[:, :ln], onesrow_sb[:],
                             bvoc_sb[:, off:off + ln],
                             start=False, stop=True)
            ex_scr = wp.tile([128, 512], BF, name=f"{R}ex{trow}_{ci}",
                             tag="ex_scr", bufs=2)
            nc.scalar.activation(ex_scr[:, :ln], ps_v[:, :ln], Act.Exp,
                                 accum_out=zc_sb[:, trow, ci:ci + 1])
        # label logit (wyT host layout [T, hl, (kt, n)] matches hT_sb)
        ey_sb = wp.tile([128, H], BF, name=f"{R}ey{trow}", tag="ey_sb", bufs=2)
        nc.vector.tensor_mul(
            ey_sb[:],
            hT_sb[:].rearrange("p k n -> p (k n)"),
            wyt_sb[:])
        ps_ll = psB.tile([1, 128], F32, name=f"{R}ps_ll{trow}",
                         tag="ps_big", bufs=1)
        for ch in range(8):   # accumulate the 8 k-tiles directly in PSUM
            nc.tensor.matmul(ps_ll[:], ones1_sb[:],
                             ey_sb[:, bass.ts(ch, 128)],
                             start=(ch == 0), stop=(ch == 7))
        ll_t = wp.tile([1, 128], F32, name=f"{R}ll_t{trow}",
                       tag="ll_t", bufs=2)
        nc.scalar.copy(ll_t[:], ps_ll[:])
        nc.sync.dma_start(
            pay2[ZB:].bitcast(F32)
            .rearrange("(n t) -> t n", n=128)[trow:trow + 1, :],
            ll_t[:])

    for t in range(T):
        # one PSUM accumulates x@Wx + b + h@Wh + attn@Wattn for this step;
        # the Wx and bias matmuls run during the AllGather wait.
        xt_sb = wp.tile([128, 4, 128], BF, name=f"{R}xt{t}", tag="xt_sb",
                        bufs=3)
        nc.sync.dma_start(xt_sb[:], xv[:, :, t, :])
        if t > 0 and "novocab" not in abl:
            wyt_sb = vocab_prefetch(t - 1)
        ps_all = psB.tile([128, SC], F32, name=f"{R}ps_all{t}", tag="ps_mm",
                          bufs=3)
        for kt in range(4):
            nc.tensor.matmul(ps_all[:], xt_sb[:, kt, :], wxs_sb[:, kt, :],
                             start=(kt == 0), stop=False)
        nc.tensor.matmul(ps_all[:], onesrow_sb[:], brow_sb[:],
                         start=False, stop=False)
        hT_sb = wp.tile([128, 8, 128], BF, name=f"{R}hT{t}", tag="hT_sb",
                        bufs=3)
        if "noag" in abl:
            for c2 in range(8):
                nc.sync.dma_start(
                    hT_sb[:, c2, :],
                    gat[0, 0:HT_BYTES].bitcast(BF)
                    .rearrange("(p b) -> p b", p=128))
        else:
            nc.sync.dma_start(
                hT_sb[:],
                gat[:, 0:HT_BYTES].bitcast(BF)
                .rearrange("c (p b) -> p c b", p=128))
        noat = "noattn" in abl
        for kt in range(8):
            nc.tensor.matmul(ps_all[:], hT_sb[:, kt, :], whs_sb[:, kt, :],
                             start=False, stop=(noat and kt == 7))
        a_sb = wp.tile([128, SC], BF, name=f"{R}a{t}", tag="a_sb")
        if not noat:
            sc_sb = wp.tile([128, 8, P], F32, name=f"{R}sc{t}", tag="sc_sb")
            if "noag" in abl:
                for c2 in range(8):
                    nc.sync.dma_start(
                        sc_sb[:, c2, :],
                        gat[0, HT_BYTES:PAY].bitcast(F32)
                        .rearrange("(n q) -> n q", n=128))
            else:
                nc.sync.dma_start(
                    sc_sb[:],
                    gat[:, HT_BYTES:PAY].bitcast(F32)
                    .rearrange("c (n q) -> n c q", n=128))
            # softmax weights (no max subtraction: |scores| < ~10)
            s4 = wp.tile([128, 4, P], F32, name=f"{R}s4_{t}", tag="s4")
            nc.vector.tensor_add(s4[:], sc_sb[:, 0:4, :], sc_sb[:, 4:8, :])
            s2 = wp.tile([128, 2, P], F32, name=f"{R}s2_{t}", tag="s2")
            nc.vector.tensor_add(s2[:], s4[:, 0:2, :], s4[:, 2:4, :])
            ssum = wp.tile([128, P], F32, name=f"{R}ssum{t}", tag="ssum")
            nc.vector.tensor_add(ssum[:], s2[:, 0, :], s2[:, 1, :])
            e_w = wp.tile([128, P], F32, name=f"{R}ew{t}", tag="e_w")
            zs = wp.tile([128, 1], F32, name=f"{R}zs{t}", tag="zs")
            nc.scalar.activation(e_w[:], ssum[:], Act.Exp, scale=1.0 / 32.0,
                                 accum_out=zs[:])
            rz = wp.tile([128, 1], F32, name=f"{R}rz{t}", tag="rz")
            nc.vector.reciprocal(rz[:], zs[:])
            # attention on PE: += sum_p diag(w[:,p]) @ B_p with normalized
            # w = e_w/z; diagonals built in 4 fused DVE ops so the PE can
            # start consuming after the first quarter.
            dga = wp.tile([128, P, 128], BF, name=f"{R}dga{t}", tag="dga",
                          bufs=1)
            for q in range(4):
                nc.vector.scalar_tensor_tensor(
                    dga[:, 4 * q:4 * q + 4, :], ident16[:, 4 * q:4 * q + 4, :],
                    rz[:],
                    e_w[:, 4 * q:4 * q + 4].unsqueeze(2)
                    .broadcast_to([128, 4, 128]),
                    op0=AluOp.mult, op1=AluOp.mult)
                for p in range(4 * q, 4 * q + 4):
                    nc.tensor.matmul(ps_all[:], dga[:, p, :], b_sb[:, p, :],
                                     start=False, stop=(p == P - 1))
        nc.scalar.copy(a_sb[:], ps_all[:])
        # gates: [i | f | o | g] chunks of 128
        sig = wp.tile([128, 384], BF, name=f"{R}sig{t}", tag="sig")
        tg = wp.tile([128, 128], BF, name=f"{R}tg{t}", tag="tg")
        nc.scalar.activation(sig[:], a_sb[:, 0:384], Act.Sigmoid)
        nc.scalar.activation(tg[:], a_sb[:, 384:512], Act.Tanh)
        fc = wp.tile([128, 128], F32, name=f"{R}fc{t}", tag="fc")
        ig = wp.tile([128, 128], F32, name=f"{R}ig{t}", tag="ig")
        nc.vector.tensor_mul(fc[:], sig[:, 128:256], c_sb[:])
        nc.vector.tensor_mul(ig[:], sig[:, 0:128], tg[:])
        nc.vector.tensor_add(c_sb[:], fc[:], ig[:])
        tc_t = wp.tile([128, 128], BF, name=f"{R}tc{t}", tag="tc_t")
        nc.scalar.activation(tc_t[:], c_sb[:], Act.Tanh)
        h_sl = wp.tile([128, 128], BF, name=f"{R}hsl{t}", tag="h_sl")
        nc.vector.tensor_mul(h_sl[:], sig[:, 256:384], tc_t[:])
        # transpose h slice (payload + next-step lhsT live in gat)
        ps_tr = psB.tile([128, 128], BF, name=f"{R}ps_tr{t}",
                         tag="ps_tr", bufs=1)
        nc.tensor.transpose(ps_tr[:], h_sl[:], ident_sb[:])
        hT_c = wp.tile([128, 128], BF, name=f"{R}hTc{t}", tag="hT_c", bufs=2)
        nc.scalar.copy(hT_c[:], ps_tr[:])
        # scores partial for h_{t+1} + AllGather
        gat_next = scores_and_ag(t + 1, hT_c[:], h_sl[:])
        # vocab row t uses hs[t] = h_{t+1}... but row t-1 = h_t is
        # what we have gathered now (hT_sb).
        if t > 0 and "novocab" not in abl:
            vocab_row(t - 1, hT_sb, wyt_sb)
        gat = gat_next

    # tail: vocab for last row (h_T from final AG)
    if "novocab" not in abl:
        hT_last = wp.tile([128, 8, 128], BF, name=f"{R}hT_last", tag="hT_sb")
        if "noag" in abl:
            for c2 in range(8):
                nc.sync.dma_start(
                    hT_last[:, c2, :],
                    gat[0, 0:HT_BYTES].bitcast(BF)
                    .rearrange("(p b) -> p b", p=128))
        else:
            nc.sync.dma_start(
                hT_last[:],
                gat[:, 0:HT_BYTES].bitcast(BF)
                .rearrange("c (p b) -> p c b", p=128))
        vocab_row(T - 1, hT_last, vocab_prefetch(T - 1))

    # ---------- final loss ----------
    if "novocab" in abl:
        nc.vector.memset(zc_sb[:], 1.0)
    zfin = wp.tile([128, T], F32, name=f"{R}zfin", tag="zfin")
    nc.vector.tensor_reduce(zfin[:], zc_sb[:], mybir.AxisListType.X,
                            AluOp.add)
    nc.sync.dma_start(
        pay2[0:ZB].bitcast(F32).rearrange("(p b) -> p b", p=128),
        zfin[:])
    if "novocab" in abl:   # fill the label-logit half too (same size)
        nc.sync.dma_start(
            pay2[ZB:].bitcast(F32).rearrange("(p b) -> p b", p=128),
            zfin[:])
    gat2 = dram.tile([NCORES, N * T * 4 + T * 128 * 4], U8, name=f"{R}gat2",
                     tag="gat2", addr_space="Shared")
    nc.gpsimd.collective_compute(
        "AllGather", AluOp.bypass, replica_groups=rg,
        ins=[pay2.opt()], outs=[gat2.opt()])
    zg = wp.tile([128, T, 8], F32, name=f"{R}zg", tag="zg")
    lg = wp.tile([128, T, 8], F32, name=f"{R}lg", tag="lg")
    for c2 in range(8):
        nc.sync.dma_start(
            zg[:, :, c2],
            gat2[c2, 0:ZB].bitcast(F32).rearrange("(n q) -> n q", n=128))
        nc.sync.dma_start(
            lg[:, :, c2],
            gat2[c2, ZB:].bitcast(F32)
            .rearrange("(n q) -> n q", n=128))
    zred = wp.tile([128, T], F32, name=f"{R}zred", tag="zred")
    llred = wp.tile([128, T], F32, name=f"{R}llred", tag="llred")
    nc.vector.tensor_reduce(zred[:], zg[:], mybir.AxisListType.X,
                            AluOp.add)
    nc.vector.tensor_reduce(llred[:], lg[:], mybir.AxisListType.X,
                            AluOp.add)
    lse = wp.tile([128, T], F32, name=f"{R}lse", tag="lse")
    nc.scalar.activation(lse[:], zred[:], Act.Ln)
    diff = wp.tile([128, T], F32, name=f"{R}diff", tag="diff")
    nc.vector.tensor_sub(diff[:], lse[:], llred[:])
    nc.vector.tensor_mul(diff[:], diff[:], mask_sb[:])
    per_n = wp.tile([128, 1], F32, name=f"{R}per_n", tag="per_n")
    nc.vector.tensor_reduce(per_n[:], diff[:], mybir.AxisListType.X,
                            AluOp.add)
    pn_red = wp.tile([128, 1], F32, name=f"{R}pn_red", tag="pn_red")
    nc.gpsimd.partition_all_reduce(pn_red[:], per_n[:], 128,
                                   bass_isa.ReduceOp.add)
    loss_sb = wp.tile([1, 1], F32, name=f"{R}loss_sb", tag="loss_sb")
    nc.scalar.mul(loss_sb[:], pn_red[0:1, :], 1.0 / N)
    nc.sync.dma_start(loss_out[:], loss_sb[:])


def host_prep(inputs, T):
    """Build the 8 per-core input maps (all numpy)."""
    g = {k: np.asarray(v) for k, v in inputs.items()}
    images, captions = g["images"], g["captions"]
    W_embed, W_proj, b_proj = g["W_embed"], g["W_proj"], g["b_proj"]
    Wx, Wh, Wattn, b = g["Wx"], g["Wh"], g["Wattn"], g["b"]
    W_vocab, b_vocab = g["W_vocab"], g["b_vocab"]

    cap = np.asarray(captions)
    cap_in = cap[:, :T]
    cap_out = cap[:, 1:T + 1]
    x_emb = W_embed[cap_in]                      # [N, T, WD]
    xembT2 = np.ascontiguousarray(               # [128, (kt, t, n)]
        x_emb.transpose(2, 1, 0).reshape(4, 128, T, N)
        .transpose(1, 0, 2, 3).reshape(128, 4 * T * N)).astype(BF16)
    imgsT = np.ascontiguousarray(                # (cin, p, n)
        images.reshape(N, CIN, P).transpose(1, 2, 0).reshape(CIN, P * N)
    ).astype(BF16)
    mask = (cap_out != 0).astype(np.float32)     # [N, T]
    ident = np.eye(128, dtype=BF16)
    ones1 = np.ones((128, 1), dtype=BF16)
    onesrow = np.ones((1, 128), dtype=BF16)

    # label weight vectors, [H, N, T] -> per t: [hl, (kt, n)]
    wy = W_vocab[:, cap_out]                     # [H, N, T]
    wy_t = wy.reshape(8, 128, N, T).transpose(3, 1, 0, 2)  # [T, hl, kt, n]

    in_maps = []
    for c in range(NCORES):
        hsl = slice(128 * c, 128 * (c + 1))
        idx = np.concatenate([g4 * H + 128 * c + np.arange(128)
                              for g4 in range(4)])
        vsl = slice(VS * c, VS * (c + 1))
        wyc = wy_t.copy()
        nm = np.zeros(N, dtype=wy_t.dtype)
        nm[16 * c:16 * (c + 1)] = 1
        wyc *= nm[None, None, None, :]
        in_maps.append({
            "imgsT": imgsT,
            "wprojS": np.ascontiguousarray(W_proj[:, hsl]).astype(BF16),
            "bprojS": np.ascontiguousarray(b_proj[hsl, None]).astype(np.float32),
            "xembT2": xembT2,
            "WxS": np.ascontiguousarray(Wx[:, idx]).astype(BF16),
            "brow": np.ascontiguousarray(b[None, idx]).astype(BF16),
            "WhS": np.ascontiguousarray(Wh[:, idx]).astype(BF16),
            "WattnS": np.ascontiguousarray(Wattn[:, idx]).astype(BF16),
            "wvoc": np.ascontiguousarray(W_vocab[:, vsl]).astype(BF16),
            "bvoc": np.ascontiguousarray(b_vocab[None, vsl]).astype(BF16),
            "wyT": np.ascontiguousarray(
                wyc.reshape(T, 128, H)).astype(BF16),
            "mask_f": mask,
            "ident": ident,
            "ones1": ones1,
            "onesrow": onesrow,
        })
    host_by = float(np.sum(mask.astype(np.float64) *
                           np.asarray(b_vocab, np.float64)[cap_out]) / N)
    return in_maps, host_by


_CACHE = {}


def _get_built(T, reps=1):
    if (T, reps) not in _CACHE:
        _CACHE[(T, reps)] = build(T, reps)
    return _CACHE[(T, reps)]


def run(inputs, T=30):
    nc = _get_built(T)
    in_maps, host_by = host_prep(inputs, T)
    res = run_bass_kernel_spmd(nc, in_maps, core_ids=list(range(NCORES)))
    dev_loss = float(res.results[0]["loss"][0, 0])
    return np.float32(dev_loss - host_by)


def kernel(**inputs) -> np.ndarray:
    return run(inputs, T=30)


# revision 52
# speedup vs baseline: 1.1109x; 1.1109x over previous
"""CaptioningRNN (LSTM + spatial attention + vocab loss) on 8 Trainium2 cores.

Strategy:
 - The three big parallel matmul groups (CNN-feature projection, x@Wx
   precompute, vocab logits + logsumexp) are sharded 8 ways.
 - The sequential LSTM is sharded over the 4H gate dimension: core c owns a
   128-wide slice of each gate (aligned with hidden slice c). Each step ends
   with a fused AllGather carrying (a) the core's transposed h-slice (feeds
   the next step's matmuls on every core) and (b) partial attention scores
   for the core's hidden slice (summed on every core after the gather).
 - Attention scores are computed batch-major directly from h_sl via a
   broadcast-multiply + reduce against A_batch[n, p, h]; attn @ Wattn is
   eliminated via the B[n, sc, p] = A[n, :, p] @ Wattn precompute and a
   single broadcast-multiply + reduce over p, with the softmax
   normalization (1/z) folded into the final fused scalar_tensor_tensor.
 - Loss: logsumexp partials per vocab shard (no max subtraction - logits are
   O(3)), label logits via host-gathered W_vocab columns, one final AllGather,
   then a replicated tiny reduction. b_vocab[y] mask term added on host.

build(T, reps=K) emits the whole computation K times inside one NEFF
(weights preloaded once, all work tiles shared across reps via tags) so the
per-execution HW time can be measured as the wall-clock slope over K,
amortizing out the fixed axon dispatch overhead.
"""
import sys, os, time

sys.path.insert(0, "/opt/trn_rl_repo")

import numpy as np
import ml_dtypes

import concourse.bass as bass
import concourse.bacc as bacc
import concourse.mybir as mybir
import concourse.tile as tile
import concourse.bass_isa as bass_isa
from concourse.bass_utils import run_bass_kernel_spmd


BF16 = ml_dtypes.bfloat16
F32 = mybir.dt.float32
BF = mybir.dt.bfloat16
U8 = mybir.dt.uint8

NCORES = 8
N = 128          # batch
TCAP = 31
CIN = 1280
WD = 512         # embed dim
H = 1024         # hidden
V = 10000        # vocab
P = 16           # spatial positions
HS = H // NCORES      # 128 hidden slice
SC = 4 * HS           # 512 a-columns per core
VS = V // NCORES      # 1250 vocab slice
AluOp = mybir.AluOpType
Act = mybir.ActivationFunctionType

HT_BYTES = 128 * 128 * 2           # h^T slice, bf16
SC_BYTES = 128 * 16 * 4            # partial scores [n, p], fp32
PAY = HT_BYTES + SC_BYTES          # per-step AG payload bytes


def build(T, reps=1, abl=()):
    nc = bacc.Bacc("TRN2", target_bir_lowering=False, debug=False,
                   num_devices=NCORES)
    NT = T * N

    def din(name, shape, dt):
        return nc.dram_tensor(name, shape, dt, kind="ExternalInput").ap()

    imgsT = din("imgsT", [CIN, P * N], BF)      # (cin, p, n) layout
    # xembT2: [128, (kt, t, n)] so one DMA loads a step's x_t k-tiles
    xembT2 = din("xembT2", [128, 4 * T * 128], BF)
    wprojS = din("wprojS", [CIN, HS], BF)
    bprojS = din("bprojS", [HS, 1], F32)
    WxS = din("WxS", [WD, SC], BF)
    brow = din("brow", [1, SC], BF)
    WhS = din("WhS", [H, SC], BF)
    WattnS = din("WattnS", [H, SC], BF)
    wvoc = din("wvoc", [H, VS], BF)
    bvoc = din("bvoc", [1, VS], BF)
    wyT = din("wyT", [T, 128, H], BF)
    mask_f = din("mask_f", [N, T], F32)
    ident = din("ident", [128, 128], BF)
    ones1 = din("ones1", [128, 1], BF)
    onesrow = din("onesrow", [1, 128], BF)

    loss_out = nc.dram_tensor("loss", [1, 1], F32, kind="ExternalOutput").ap()

    VCH = [(0, 512), (512, 512), (1024, VS - 1024)]  # vocab chunks
    ZB = N * T * 4
    rg = [list(range(NCORES))]

    with tile.TileContext(nc) as tc:
        with (
            tc.tile_pool(name="dram", bufs=1, space="DRAM") as dram,
            tc.tile_pool(name="persist", bufs=1) as pp,
            tc.tile_pool(name="work", bufs=3) as wp,
            tc.tile_pool(name="psB", bufs=1, space="PSUM") as psB,
        ):
            # ---------- persistent SBUF: weights, loaded once ----------
            whs_sb = pp.tile([128, 8, SC], BF, name="whs_sb")
            wvoc_sb = pp.tile([128, 8, VS], BF, name="wvoc_sb")
            bvoc_sb = pp.tile([1, VS], BF, name="bvoc_sb")
            ident_sb = pp.tile([128, 128], BF, name="ident_sb")
            ones1_sb = pp.tile([128, 1], BF, name="ones1_sb")
            onesrow_sb = pp.tile([1, 128], BF, name="onesrow_sb")
            bproj_sb = pp.tile([HS, 1], F32, name="bproj_sb")
            mask_sb = pp.tile([N, T], F32, name="mask_sb")
            wproj_sb = pp.tile([128, 10, 128], BF, name="wproj_sb")
            watt_sb = pp.tile([128, 8, SC], BF, name="watt_sb")
            wxs_sb = pp.tile([128, 4, SC], BF, name="wxs_sb")
            brow_sb = pp.tile([1, SC], BF, name="brow_sb")

            for dst, src in [(ident_sb, ident), (ones1_sb, ones1),
                             (onesrow_sb, onesrow),
                             (bproj_sb, bprojS),
                             (mask_sb, mask_f), (bvoc_sb, bvoc),
                             (brow_sb, brow)]:
                nc.sync.dma_start(dst[:], src[:])
            for kt in range(8):
                nc.sync.dma_start(whs_sb[:, kt, :], WhS[bass.ts(kt, 128), :])
                nc.sync.dma_start(wvoc_sb[:, kt, :], wvoc[bass.ts(kt, 128), :])
                nc.sync.dma_start(watt_sb[:, kt, :], WattnS[bass.ts(kt, 128), :])
            for kt in range(10):
                nc.sync.dma_start(wproj_sb[:, kt, :], wprojS[bass.ts(kt, 128), :])
            for kt in range(4):
                nc.sync.dma_start(wxs_sb[:, kt, :], WxS[bass.ts(kt, 128), :])

            for rep in range(reps):
                emit(nc, tc, dram, wp, psB, rep, T,
                     dict(imgsT=imgsT, xembT2=xembT2, wyT=wyT,
                          loss_out=loss_out,
                          whs_sb=whs_sb, wvoc_sb=wvoc_sb, bvoc_sb=bvoc_sb,
                          ident_sb=ident_sb,
                          ones1_sb=ones1_sb, onesrow_sb=onesrow_sb,
                          bproj_sb=bproj_sb, mask_sb=mask_sb,
                          wproj_sb=wproj_sb, watt_sb=watt_sb,
                          wxs_sb=wxs_sb, brow_sb=brow_sb),
                     VCH, ZB, rg, abl)

    nc.compile()
    return nc


def emit(nc, tc, dram, wp, psB, rep, T, g, VCH, ZB, rg, abl=()):
    """One full forward pass; work tiles share buffers across reps via tags.

    abl: ablation flags for timing experiments only (breaks correctness):
      "novocab" - skip per-row vocab logsumexp work
      "noattn"  - skip score/softmax/attention path
      "noag"    - replace the per-step AllGather with local copies
    """
    R = f"r{rep}"
    imgsT, xembT2, wyT = g["imgsT"], g["xembT2"], g["wyT"]
    loss_out = g["loss_out"]
    whs_sb, wvoc_sb, bvoc_sb = g["whs_sb"], g["wvoc_sb"], g["bvoc_sb"]
    ident_sb = g["ident_sb"]
    ones1_sb, onesrow_sb = g["ones1_sb"], g["onesrow_sb"]
    bproj_sb, mask_sb = g["bproj_sb"], g["mask_sb"]
    wproj_sb, watt_sb = g["wproj_sb"], g["watt_sb"]
    wxs_sb, brow_sb = g["wxs_sb"], g["brow_sb"]

    # per-rep state tiles (shared buffers across reps via tags)
    ident16 = wp.tile([128, P, 128], BF, name=f"{R}ident16", tag="ident16",
                      bufs=1)
    for p in range(P):
        nc.scalar.copy(ident16[:, p, :], g["ident_sb"][:])
    atm = wp.tile([128, P, N], BF, name=f"{R}atm", tag="atm", bufs=1)
    ab = wp.tile([128, P, HS], BF, name=f"{R}ab", tag="ab", bufs=1)  # A[n,p,h]
    at_all = wp.tile([128, 8, P, N], BF, name=f"{R}at_all", tag="at_all", bufs=1)
    b_sb = wp.tile([128, P, SC], BF, name=f"{R}b_sb", tag="b_sb", bufs=1)
    zc_sb = wp.tile([128, T, 3], F32, name=f"{R}zc_sb", tag="zc_sb", bufs=1)
    c_sb = wp.tile([128, 128], F32, name=f"{R}c_sb", tag="c_sb", bufs=1)

    # ---------- P1: projection  A^T[my hslice, (p,n)] ----------
    ps_a = psB.tile([128, P * N], F32, name=f"{R}ps_a", tag="ps_big")
    for kt in range(10):
        imgs_kt = wp.tile([128, P * N], BF, name=f"{R}imgs{kt}",
                          tag="imgs_kt", bufs=2)
        nc.sync.dma_start(imgs_kt[:], imgsT[bass.ts(kt, 128), :])
        for ch in range(4):
            nc.tensor.matmul(ps_a[:, bass.ts(ch, 512)],
                             wproj_sb[:, kt, :],
                             imgs_kt[:, bass.ts(ch, 512)],
                             start=(kt == 0), stop=(kt == 9))
    # + b_proj (per-partition), cast bf16
    nc.scalar.activation(atm[:].rearrange("p n q -> p (n q)"),
                         ps_a[:], Act.Identity, bias=bproj_sb[:])
    # A batch-major copy: ab[n, p, h] via 16 PE transposes of atm[:, p, :]
    for p in range(P):
        ps_t2 = psB.tile([128, 128], BF, name=f"{R}ps_ab{p}", tag="ps_tr",
                         bufs=1)
        nc.tensor.transpose(ps_t2[:], atm[:, p, :], ident_sb[:])
        nc.scalar.copy(ab[:, p, :], ps_t2[:])
    # h0^T my slice = mean over positions (strided view [h, n, p])
    h0t_f = wp.tile([128, 128], F32, name=f"{R}h0t_f", tag="h0t_f", bufs=1)
    nc.vector.tensor_reduce(h0t_f[:], atm[:].transpose([0, 2, 1]),
                            mybir.AxisListType.X, AluOp.add)
    nc.scalar.mul(h0t_f[:], h0t_f[:], 1.0 / P)
    h0t_b = wp.tile([128, 128], BF, name=f"{R}h0t_b", tag="h0t_b", bufs=1)
    nc.scalar.copy(h0t_b[:], h0t_f[:])
    # c0 = h0 (batch-major my hidden slice)
    ps_tr0 = psB.tile([128, 128], BF, name=f"{R}ps_tr", tag="ps_tr",
                      bufs=1)
    nc.tensor.transpose(ps_tr0[:], h0t_b[:], ident_sb[:])
    nc.scalar.copy(c_sb[:], ps_tr0[:])
    c_bf = wp.tile([128, 128], BF, name=f"{R}c_bf", tag="c_bf", bufs=1)
    nc.scalar.copy(c_bf[:], ps_tr0[:])

    # scores partial [n, p] = sum_h h[n,h] * A[n,p,h]  (scale 1/32 folded
    # into the post-AG exp), packed with h^T into the per-step AllGather.
    def scores_and_ag(step, hT_ap, hb_ap):
        pay = dram.tile([PAY], U8, name=f"{R}pay{step}", tag="pay", bufs=3)
        if "noattn" not in abl:
            e_b = wp.tile([128, P, HS], BF, name=f"{R}e_{step}", tag="e_b",
                          bufs=2)
            nc.vector.tensor_mul(
                e_b[:], ab[:],
                hb_ap.unsqueeze(1).broadcast_to([128, P, HS]))
            sc_part = wp.tile([128, P], F32, name=f"{R}scp{step}",
                              tag="sc_part", bufs=2)
            nc.vector.tensor_reduce(sc_part[:], e_b[:],
                                    mybir.AxisListType.X, AluOp.add)
            nc.sync.dma_start(
                pay[HT_BYTES:PAY].bitcast(F32)
                .rearrange("(n q) -> n q", n=128),
                sc_part[:])
        nc.sync.dma_start(
            pay[0:HT_BYTES].rearrange("(p b) -> p b", p=128),
            hT_ap.bitcast(mybir.dt.uint8))
        gat = dram.tile([NCORES, PAY], U8, name=f"{R}gat{step}", tag="gat",
                        bufs=3, addr_space="Shared")
        if "noag" in abl:
            nc.sync.dma_start(gat[0], pay[:])
        else:
            nc.gpsimd.collective_compute(
                "AllGather", AluOp.bypass, replica_groups=rg,
                ins=[pay.opt()], outs=[gat.opt()])
        return gat

    gat = scores_and_ag(0, h0t_b[:], c_bf[:])

    # ---------- at_all: gather A^T from all cores (two halves, each
    # half's B-precompute overlaps the next half's AllGather) ----------
    HP = P * N // 2
    gavs = []
    for hf in range(2):
        pay_a = dram.tile([128, HP], BF, name=f"{R}pay_a{hf}",
                          tag=f"pay_a{hf}")
        nc.sync.dma_start(
            pay_a[:],
            atm[:, 8 * hf:8 * hf + 8, :].rearrange("p n q -> p (n q)"))
        gat_a = dram.tile([NCORES * 128, HP], BF, name=f"{R}gat_a{hf}",
                          tag=f"gat_a{hf}", addr_space="Shared")
        nc.gpsimd.collective_compute(
            "AllGather", AluOp.bypass, replica_groups=rg,
            ins=[pay_a.opt()], outs=[gat_a.opt()])
        gavs.append(gat_a[:].rearrange("(c p) f -> c p f", c=NCORES))

    # ---------- B precompute: b_sb[n, p, :] = A[n, :, p] @ WattnS ----------
    for hf in range(2):
        for kt in range(8):
            nc.sync.dma_start(
                at_all[:, kt, 8 * hf:8 * hf + 8, :]
                .rearrange("p n q -> p (n q)"), gavs[hf][kt])
        for p in range(8 * hf, 8 * hf + 8):
            ps_b = psB.tile([128, SC], F32, name=f"{R}ps_b{p}", tag="ps_mm",
                            bufs=3)
            for kt in range(8):
                nc.tensor.matmul(ps_b[:], at_all[:, kt, p, :],
                                 watt_sb[:, kt, :],
                                 start=(kt == 0), stop=(kt == 7))
            nc.scalar.copy(b_sb[:, p, :], ps_b[:])

    # ---------- recurrence ----------
    xv = xembT2[:].rearrange("p (k t n) -> p k t n", k=4, t=T)
    pay2 = dram.tile([N * T * 4 + T * 128 * 4], U8, name=f"{R}pay2", tag="pay2")

    def vocab_prefetch(trow):
        wyt_sb = wp.tile([128, H], BF, name=f"{R}wyt{trow}", tag="wyt_sb",
                         bufs=2)
        nc.sync.dma_start(wyt_sb[:], wyT[trow])
        return wyt_sb

    def vocab_row(trow, hT_sb, wyt_sb):
        # logsumexp partials + label logits for hs row `trow` (h_{trow+1})
        for ci, (off, ln) in enumerate(VCH):
            ps_v = psB.tile([128, 512], F32, name=f"{R}ps_v{trow}_{ci}",
                            tag="ps_mm", bufs=3)
            for kt in range(8):
                nc.tensor.matmul(ps_v[:, :ln], hT_sb[:, kt, :],
                                 wvoc_sb[:, kt, off:off + ln],
                                 start=(kt == 0), stop=False)
            nc.tensor.matmul(ps_v[:, :ln], onesrow_sb[:],
                             bvoc_sb[:, off:off + ln],
                             start=False, stop=True)
            ex_scr = wp.tile([128, 512], BF, name=f"{R}ex{trow}_{ci}",
                             tag="ex_scr", bufs=2)
            nc.scalar.activation(ex_scr[:, :ln], ps_v[:, :ln], Act.Exp,
                                 accum_out=zc_sb[:, trow, ci:ci + 1])
        # label logit (wyT host layout [T, hl, (kt, n)] matches hT_sb)
        ey_sb = wp.tile([128, H], BF, name=f"{R}ey{trow}", tag="ey_sb", bufs=2)
        nc.vector.tensor_mul(
            ey_sb[:],
            hT_sb[:].rearrange("p k n -> p (k n)"),
            wyt_sb[:])
        ps_ll = psB.tile([1, 128], F32, name=f"{R}ps_ll{trow}",
                         tag="ps_big", bufs=1)
        for ch in range(8):   # accumulate the 8 k-tiles directly in PSUM
            nc.tensor.matmul(ps_ll[:], ones1_sb[:],
                             ey_sb[:, bass.ts(ch, 128)],
                             start=(ch == 0), stop=(ch == 7))
        ll_t = wp.tile([1, 128], F32, name=f"{R}ll_t{trow}",
                       tag="ll_t", bufs=2)
        nc.scalar.copy(ll_t[:], ps_ll[:])
        nc.sync.dma_start(
            pay2[ZB:].bitcast(F32)
            .rearrange("(n t) -> t n", n=128)[trow:trow + 1, :],
            ll_t[:])

    for t in range(T):
        # one PSUM accumulates x@Wx + b + h@Wh + attn@Wattn for this step;
        # the Wx and bias matmuls run during the AllGather wait.
        xt_sb = wp.tile([128, 4, 128], BF, name=f"{R}xt{t}", tag="xt_sb",
                        bufs=3)
        nc.sync.dma_start(xt_sb[:], xv[:, :, t, :])
        if t > 0 and "novocab" not in abl:
            wyt_sb = vocab_prefetch(t - 1)
        ps_all = psB.tile([128, SC], F32, name=f"{R}ps_all{t}", tag="ps_mm",
                          bufs=3)
        for kt in range(4):
            nc.tensor.matmul(ps_all[:], xt_sb[:, kt, :], wxs_sb[:, kt, :],
                             start=(kt == 0), stop=False)
        nc.tensor.matmul(ps_all[:], onesrow_sb[:], brow_sb[:],
                         start=False, stop=False)
        hT_sb = wp.tile([128, 8, 128], BF, name=f"{R}hT{t}", tag="hT_sb",
                        bufs=3)
        noat = "noattn" in abl
        if "noag" in abl:
            for c2 in range(8):
                nc.sync.dma_start(
                    hT_sb[:, c2, :],
                    gat[0, 0:HT_BYTES].bitcast(BF)
                    .rearrange("(p b) -> p b", p=128))
            for kt in range(8):
                nc.tensor.matmul(ps_all[:], hT_sb[:, kt, :], whs_sb[:, kt, :],
                                 start=False, stop=(noat and kt == 7))
        else:
            # unpack in two halves so Wh matmuls start after the first
            gv = gat[:, 0:HT_BYTES].bitcast(BF).rearrange(
                "c (p b) -> p c b", p=128)
            for hf in range(2):
                nc.sync.dma_start(hT_sb[:, 4 * hf:4 * hf + 4, :],
                                  gv[:, 4 * hf:4 * hf + 4, :])
                for kt in range(4 * hf, 4 * hf + 4):
                    nc.tensor.matmul(ps_all[:], hT_sb[:, kt, :],
                                     whs_sb[:, kt, :],
                                     start=False, stop=(noat and kt == 7))
        if not noat:
            sc_sb = wp.tile([128, 8, P], F32, name=f"{R}sc{t}", tag="sc_sb")
            if "noag" in abl:
                for c2 in range(8):
                    nc.sync.dma_start(
                        sc_sb[:, c2, :],
                        gat[0, HT_BYTES:PAY].bitcast(F32)
                        .rearrange("(n q) -> n q", n=128))
            else:
                nc.sync.dma_start(
                    sc_sb[:],
                    gat[:, HT_BYTES:PAY].bitcast(F32)
                    .rearrange("c (n q) -> n c q", n=128))
            # softmax weights (no max subtraction: |scores| < ~10)
            s4 = wp.tile([128, 4, P], F32, name=f"{R}s4_{t}", tag="s4")
            nc.vector.tensor_add(s4[:], sc_sb[:, 0:4, :], sc_sb[:, 4:8, :])
            s2 = wp.tile([128, 2, P], F32, name=f"{R}s2_{t}", tag="s2")
            nc.vector.tensor_add(s2[:], s4[:, 0:2, :], s4[:, 2:4, :])
            ssum = wp.tile([128, P], F32, name=f"{R}ssum{t}", tag="ssum")
            nc.vector.tensor_add(ssum[:], s2[:, 0, :], s2[:, 1, :])
            e_w = wp.tile([128, P], F32, name=f"{R}ew{t}", tag="e_w")
            zs = wp.tile([128, 1], F32, name=f"{R}zs{t}", tag="zs")
            nc.scalar.activation(e_w[:], ssum[:], Act.Exp, scale=1.0 / 32.0,
                                 accum_out=zs[:])
            rz = wp.tile([128, 1], F32, name=f"{R}rz{t}", tag="rz")
            nc.vector.reciprocal(rz[:], zs[:])
            # attention on PE: += sum_p diag(w[:,p]) @ B_p with normalized
            # w = e_w/z; diagonals built in 4 fused DVE ops so the PE can
            # start consuming after the first quarter.
            dga = wp.tile([128, P, 128], BF, name=f"{R}dga{t}", tag="dga",
                          bufs=1)
            for q in range(4):
                nc.vector.scalar_tensor_tensor(
                    dga[:, 4 * q:4 * q + 4, :], ident16[:, 4 * q:4 * q + 4, :],
                    rz[:],
                    e_w[:, 4 * q:4 * q + 4].unsqueeze(2)
                    .broadcast_to([128, 4, 128]),
                    op0=AluOp.mult, op1=AluOp.mult)
                for p in range(4 * q, 4 * q + 4):
                    nc.tensor.matmul(ps_all[:], dga[:, p, :], b_sb[:, p, :],
                                     start=False, stop=(p == P - 1))
        # gates: [i | f | o | g] chunks of 128, read from PSUM directly
        sig = wp.tile([128, 384], BF, name=f"{R}sig{t}", tag="sig")
        tg = wp.tile([128, 128], BF, name=f"{R}tg{t}", tag="tg")
        nc.scalar.activation(sig[:], ps_all[:, 0:384], Act.Sigmoid)
        nc.scalar.activation(tg[:], ps_all[:, 384:512], Act.Tanh)
        fc = wp.tile([128, 128], F32, name=f"{R}fc{t}", tag="fc")
        ig = wp.tile([128, 128], F32, name=f"{R}ig{t}", tag="ig")
        nc.vector.tensor_mul(fc[:], sig[:, 128:256], c_sb[:])
        nc.vector.tensor_mul(ig[:], sig[:, 0:128], tg[:])
        nc.vector.tensor_add(c_sb[:], fc[:], ig[:])
        tc_t = wp.tile([128, 128], BF, name=f"{R}tc{t}", tag="tc_t")
        nc.scalar.activation(tc_t[:], c_sb[:], Act.Tanh)
        h_sl = wp.tile([128, 128], BF, name=f"{R}hsl{t}", tag="h_sl")
        nc.vector.tensor_mul(h_sl[:], sig[:, 256:384], tc_t[:])
        # transpose h slice (payload + next-step lhsT live in gat)
        ps_tr = psB.tile([128, 128], BF, name=f"{R}ps_tr{t}",
                         tag="ps_tr", bufs=1)
        nc.tensor.transpose(ps_tr[:], h_sl[:], ident_sb[:])
        hT_c = wp.tile([128, 128], BF, name=f"{R}hTc{t}", tag="hT_c", bufs=2)
        nc.scalar.copy(hT_c[:], ps_tr[:])
        # scores partial for h_{t+1} + AllGather
        gat_next = scores_and_ag(t + 1, hT_c[:], h_sl[:])
        # vocab row t uses hs[t] = h_{t+1}... but row t-1 = h_t is
        # what we have gathered now (hT_sb).
        if t > 0 and "novocab" not in abl:
            vocab_row(t - 1, hT_sb, wyt_sb)
        gat = gat_next

    # tail: vocab for last row (h_T from final AG)
    if "novocab" not in abl:
        hT_last = wp.tile([128, 8, 128], BF, name=f"{R}hT_last", tag="hT_sb")
        if "noag" in abl:
            for c2 in range(8):
                nc.sync.dma_start(
                    hT_last[:, c2, :],
                    gat[0, 0:HT_BYTES].bitcast(BF)
                    .rearrange("(p b) -> p b", p=128))
        else:
            nc.sync.dma_start(
                hT_last[:],
                gat[:, 0:HT_BYTES].bitcast(BF)
                .rearrange("c (p b) -> p c b", p=128))
        vocab_row(T - 1, hT_last, vocab_prefetch(T - 1))

    # ---------- final loss ----------
    if "novocab" in abl:
        nc.vector.memset(zc_sb[:], 1.0)
    zfin = wp.tile([128, T], F32, name=f"{R}zfin", tag="zfin")
    nc.vector.tensor_reduce(zfin[:], zc_sb[:], mybir.AxisListType.X,
                            AluOp.add)
    nc.sync.dma_start(
        pay2[0:ZB].bitcast(F32).rearrange("(p b) -> p b", p=128),
        zfin[:])
    if "novocab" in abl:   # fill the label-logit half too (same size)
        nc.sync.dma_start(
            pay2[ZB:].bitcast(F32).rearrange("(p b) -> p b", p=128),
            zfin[:])
    gat2 = dram.tile([NCORES, N * T * 4 + T * 128 * 4], U8, name=f"{R}gat2",
                     tag="gat2", addr_space="Shared")
    nc.gpsimd.collective_compute(
        "AllGather", AluOp.bypass, replica_groups=rg,
        ins=[pay2.opt()], outs=[gat2.opt()])
    zg = wp.tile([128, T, 8], F32, name=f"{R}zg", tag="zg")
    lg = wp.tile([128, T, 8], F32, name=f"{R}lg", tag="lg")
    for c2 in range(8):
        nc.sync.dma_start(
            zg[:, :, c2],
            gat2[c2, 0:ZB].bitcast(F32).rearrange("(n q) -> n q", n=128))
        nc.sync.dma_start(
            lg[:, :, c2],
            gat2[c2, ZB:].bitcast(F32)
            .rearrange("(n q) -> n q", n=128))
    zred = wp.tile([128, T], F32, name=f"{R}zred", tag="zred")
    llred = wp.tile([128, T], F32, name=f"{R}llred", tag="llred")
    nc.vector.tensor_reduce(zred[:], zg[:], mybir.AxisListType.X,
                            AluOp.add)
    nc.vector.tensor_reduce(llred[:], lg[:], mybir.AxisListType.X,
                            AluOp.add)
    lse = wp.tile([128, T], F32, name=f"{R}lse", tag="lse")
    nc.scalar.activation(lse[:], zred[:], Act.Ln)
    diff = wp.tile([128, T], F32, name=f"{R}diff", tag="diff")
    nc.vector.tensor_sub(diff[:], lse[:], llred[:])
    nc.vector.tensor_mul(diff[:], diff[:], mask_sb[:])
    per_n = wp.tile([128, 1], F32, name=f"{R}per_n", tag="per_n")
    nc.vector.tensor_reduce(per_n[:], diff[:], mybir.AxisListType.X,
                            AluOp.add)
    pn_red = wp.tile([128, 1], F32, name=f"{R}pn_red", tag="pn_red")
    nc.gpsimd.partition_all_reduce(pn_red[:], per_n[:], 128,
                                   bass_isa.ReduceOp.add)
    loss_sb = wp.tile([1, 1], F32, name=f"{R}loss_sb", tag="loss_sb")
    nc.scalar.mul(loss_sb[:], pn_red[0:1, :], 1.0 / N)
    nc.sync.dma_start(loss_out[:], loss_sb[:])


def host_prep(inputs, T):
    """Build the 8 per-core input maps (all numpy)."""
    g = {k: np.asarray(v) for k, v in inputs.items()}
    images, captions = g["images"], g["captions"]
    W_embed, W_proj, b_proj = g["W_embed"], g["W_proj"], g["b_proj"]
    Wx, Wh, Wattn, b = g["Wx"], g["Wh"], g["Wattn"], g["b"]
    W_vocab, b_vocab = g["W_vocab"], g["b_vocab"]

    cap = np.asarray(captions)
    cap_in = cap[:, :T]
    cap_out = cap[:, 1:T + 1]
    x_emb = W_embed[cap_in]                      # [N, T, WD]
    xembT2 = np.ascontiguousarray(               # [128, (kt, t, n)]
        x_emb.transpose(2, 1, 0).reshape(4, 128, T, N)
        .transpose(1, 0, 2, 3).reshape(128, 4 * T * N)).astype(BF16)
    imgsT = np.ascontiguousarray(                # (cin, p, n)
        images.reshape(N, CIN, P).transpose(1, 2, 0).reshape(CIN, P * N)
    ).astype(BF16)
    mask = (cap_out != 0).astype(np.float32)     # [N, T]
    ident = np.eye(128, dtype=BF16)
    ones1 = np.ones((128, 1), dtype=BF16)
    onesrow = np.ones((1, 128), dtype=BF16)

    # label weight vectors, [H, N, T] -> per t: [hl, (kt, n)]
    wy = W_vocab[:, cap_out]                     # [H, N, T]
    wy_t = wy.reshape(8, 128, N, T).transpose(3, 1, 0, 2)  # [T, hl, kt, n]

    in_maps = []
    for c in range(NCORES):
        hsl = slice(128 * c, 128 * (c + 1))
        idx = np.concatenate([g4 * H + 128 * c + np.arange(128)
                              for g4 in range(4)])
        vsl = slice(VS * c, VS * (c + 1))
        wyc = wy_t.copy()
        nm = np.zeros(N, dtype=wy_t.dtype)
        nm[16 * c:16 * (c + 1)] = 1
        wyc *= nm[None, None, None, :]
        in_maps.append({
            "imgsT": imgsT,
            "wprojS": np.ascontiguousarray(W_proj[:, hsl]).astype(BF16),
            "bprojS": np.ascontiguousarray(b_proj[hsl, None]).astype(np.float32),
            "xembT2": xembT2,
            "WxS": np.ascontiguousarray(Wx[:, idx]).astype(BF16),
            "brow": np.ascontiguousarray(b[None, idx]).astype(BF16),
            "WhS": np.ascontiguousarray(Wh[:, idx]).astype(BF16),
            "WattnS": np.ascontiguousarray(Wattn[:, idx]).astype(BF16),
            "wvoc": np.ascontiguousarray(W_vocab[:, vsl]).astype(BF16),
            "bvoc": np.ascontiguousarray(b_vocab[None, vsl]).astype(BF16),
            "wyT": np.ascontiguousarray(
                wyc.reshape(T, 128, H)).astype(BF16),
            "mask_f": mask,
            "ident": ident,
            "ones1": ones1,
            "onesrow": onesrow,
        })
    host_by = float(np.sum(mask.astype(np.float64) *
                           np.asarray(b_vocab, np.float64)[cap_out]) / N)
    return in_maps, host_by


_CACHE = {}


def _get_built(T, reps=1):
    if (T, reps) not in _CACHE:
        _CACHE[(T, reps)] = build(T, reps)
    return _CACHE[(T, reps)]


def run(inputs, T=30):
    nc = _get_built(T)
    in_maps, host_by = host_prep(inputs, T)
    res = run_bass_kernel_spmd(nc, in_maps, core_ids=list(range(NCORES)))
    dev_loss = float(res.results[0]["loss"][0, 0])
    return np.float32(dev_loss - host_by)


def kernel(**inputs) -> np.ndarray:
    return run(inputs, T=30)


# revision 53
# speedup vs baseline: 1.1449x; 1.0307x over previous
"""CaptioningRNN (LSTM + spatial attention + vocab loss) on 8 Trainium2 cores.

Strategy:
 - The three big parallel matmul groups (CNN-feature projection, x@Wx
   precompute, vocab logits + logsumexp) are sharded 8 ways.
 - The sequential LSTM is sharded over the 4H gate dimension: core c owns a
   128-wide slice of each gate (aligned with hidden slice c). Each step ends
   with a fused AllGather carrying (a) the core's transposed h-slice (feeds
   the next step's matmuls on every core) and (b) partial attention scores
   for the core's hidden slice (summed on every core after the gather).
 - Attention scores are computed batch-major directly from h_sl via a
   broadcast-multiply + reduce against A_batch[n, p, h]; attn @ Wattn is
   eliminated via the B[n, sc, p] = A[n, :, p] @ Wattn precompute and a
   single broadcast-multiply + reduce over p, with the softmax
   normalization (1/z) folded into the final fused scalar_tensor_tensor.
 - Loss: logsumexp partials per vocab shard (no max subtraction - logits are
   O(3)), label logits via host-gathered W_vocab columns, one final AllGather,
   then a replicated tiny reduction. b_vocab[y] mask term added on host.

build(T, reps=K) emits the whole computation K times inside one NEFF
(weights preloaded once, all work tiles shared across reps via tags) so the
per-execution HW time can be measured as the wall-clock slope over K,
amortizing out the fixed axon dispatch overhead.
"""
import sys, os, time

sys.path.insert(0, "/opt/trn_rl_repo")

import numpy as np
import ml_dtypes

import concourse.bass as bass
import concourse.bacc as bacc
import concourse.mybir as mybir
import concourse.tile as tile
import concourse.bass_isa as bass_isa
from concourse.bass_utils import run_bass_kernel_spmd


BF16 = ml_dtypes.bfloat16
F32 = mybir.dt.float32
BF = mybir.dt.bfloat16
U8 = mybir.dt.uint8

NCORES = 8
N = 128          # batch
TCAP = 31
CIN = 1280
WD = 512         # embed dim
H = 1024         # hidden
V = 10000        # vocab
P = 16           # spatial positions
HS = H // NCORES      # 128 hidden slice
SC = 4 * HS           # 512 a-columns per core
VS = V // NCORES      # 1250 vocab slice
AluOp = mybir.AluOpType
Act = mybir.ActivationFunctionType

HT_BYTES = 128 * 128 * 2           # h^T slice, bf16
SC_BYTES = 128 * 16 * 4            # partial scores [n, p], fp32
PAY = HT_BYTES + SC_BYTES          # per-step AG payload bytes


def build(T, reps=1, abl=()):
    nc = bacc.Bacc("TRN2", target_bir_lowering=False, debug=False,
                   num_devices=NCORES)
    NT = T * N

    def din(name, shape, dt):
        return nc.dram_tensor(name, shape, dt, kind="ExternalInput").ap()

    imgsT = din("imgsT", [CIN, P * N], BF)      # (cin, p, n) layout
    # xembT2: [128, (kt, t, n)] so one DMA loads a step's x_t k-tiles
    xembT2 = din("xembT2", [128, 4 * T * 128], BF)
    wprojS = din("wprojS", [CIN, HS], BF)
    bprojS = din("bprojS", [HS, 1], F32)
    WxS = din("WxS", [WD, SC], BF)
    brow = din("brow", [1, SC], BF)
    WhS = din("WhS", [H, SC], BF)
    WattnS = din("WattnS", [H, SC], BF)
    wvoc = din("wvoc", [H, VS], BF)
    bvoc = din("bvoc", [1, VS], BF)
    wyT = din("wyT", [T, 128, H], BF)
    mask_f = din("mask_f", [N, T], F32)
    ident = din("ident", [128, 128], BF)
    ones1 = din("ones1", [128, 1], BF)
    onesrow = din("onesrow", [1, 128], BF)

    loss_out = nc.dram_tensor("loss", [1, 1], F32, kind="ExternalOutput").ap()

    VCH = [(0, 512), (512, 512), (1024, VS - 1024)]  # vocab chunks
    ZB = N * T * 4
    rg = [list(range(NCORES))]

    with tile.TileContext(nc) as tc:
        with (
            tc.tile_pool(name="dram", bufs=1, space="DRAM") as dram,
            tc.tile_pool(name="persist", bufs=1) as pp,
            tc.tile_pool(name="work", bufs=3) as wp,
            tc.tile_pool(name="psB", bufs=1, space="PSUM") as psB,
        ):
            # ---------- persistent SBUF: weights, loaded once ----------
            whs_sb = pp.tile([128, 8, SC], BF, name="whs_sb")
            wvoc_sb = pp.tile([128, 8, VS], BF, name="wvoc_sb")
            bvoc_sb = pp.tile([1, VS], BF, name="bvoc_sb")
            ident_sb = pp.tile([128, 128], BF, name="ident_sb")
            ones1_sb = pp.tile([128, 1], BF, name="ones1_sb")
            onesrow_sb = pp.tile([1, 128], BF, name="onesrow_sb")
            bproj_sb = pp.tile([HS, 1], F32, name="bproj_sb")
            mask_sb = pp.tile([N, T], F32, name="mask_sb")
            wproj_sb = pp.tile([128, 10, 128], BF, name="wproj_sb")
            watt_sb = pp.tile([128, 8, SC], BF, name="watt_sb")
            wxs_sb = pp.tile([128, 4, SC], BF, name="wxs_sb")
            brow_sb = pp.tile([1, SC], BF, name="brow_sb")

            for dst, src in [(ident_sb, ident), (ones1_sb, ones1),
                             (onesrow_sb, onesrow),
                             (bproj_sb, bprojS),
                             (mask_sb, mask_f), (bvoc_sb, bvoc),
                             (brow_sb, brow)]:
                nc.sync.dma_start(dst[:], src[:])
            for kt in range(8):
                nc.sync.dma_start(whs_sb[:, kt, :], WhS[bass.ts(kt, 128), :])
                nc.sync.dma_start(wvoc_sb[:, kt, :], wvoc[bass.ts(kt, 128), :])
                nc.sync.dma_start(watt_sb[:, kt, :], WattnS[bass.ts(kt, 128), :])
            for kt in range(10):
                nc.sync.dma_start(wproj_sb[:, kt, :], wprojS[bass.ts(kt, 128), :])
            for kt in range(4):
                nc.sync.dma_start(wxs_sb[:, kt, :], WxS[bass.ts(kt, 128), :])

            for rep in range(reps):
                emit(nc, tc, dram, wp, psB, rep, T,
                     dict(imgsT=imgsT, xembT2=xembT2, wyT=wyT,
                          loss_out=loss_out,
                          whs_sb=whs_sb, wvoc_sb=wvoc_sb, bvoc_sb=bvoc_sb,
                          ident_sb=ident_sb,
                          ones1_sb=ones1_sb, onesrow_sb=onesrow_sb,
                          bproj_sb=bproj_sb, mask_sb=mask_sb,
                          wproj_sb=wproj_sb, watt_sb=watt_sb,
                          wxs_sb=wxs_sb, brow_sb=brow_sb),
                     VCH, ZB, rg, abl)

    nc.compile()
    return nc


def emit(nc, tc, dram, wp, psB, rep, T, g, VCH, ZB, rg, abl=()):
    """One full forward pass; work tiles share buffers across reps via tags.

    abl: ablation flags for timing experiments only (breaks correctness):
      "novocab" - skip per-row vocab logsumexp work
      "noattn"  - skip score/softmax/attention path
      "noag"    - replace the per-step AllGather with local copies
    """
    R = f"r{rep}"
    imgsT, xembT2, wyT = g["imgsT"], g["xembT2"], g["wyT"]
    loss_out = g["loss_out"]
    whs_sb, wvoc_sb, bvoc_sb = g["whs_sb"], g["wvoc_sb"], g["bvoc_sb"]
    ident_sb = g["ident_sb"]
    ones1_sb, onesrow_sb = g["ones1_sb"], g["onesrow_sb"]
    bproj_sb, mask_sb = g["bproj_sb"], g["mask_sb"]
    wproj_sb, watt_sb = g["wproj_sb"], g["watt_sb"]
    wxs_sb, brow_sb = g["wxs_sb"], g["brow_sb"]

    # per-rep state tiles (shared buffers across reps via tags)
    ident16 = wp.tile([128, P, 128], BF, name=f"{R}ident16", tag="ident16",
                      bufs=1)
    for p in range(P):
        nc.scalar.copy(ident16[:, p, :], g["ident_sb"][:])
    atm = wp.tile([128, P, N], BF, name=f"{R}atm", tag="atm", bufs=1)
    ab = wp.tile([128, P, HS], BF, name=f"{R}ab", tag="ab", bufs=1)  # A[n,p,h]
    at_all = wp.tile([128, 8, P, N], BF, name=f"{R}at_all", tag="at_all", bufs=1)
    b_sb = wp.tile([128, P, SC], BF, name=f"{R}b_sb", tag="b_sb", bufs=1)
    zc_sb = wp.tile([128, T, 3], F32, name=f"{R}zc_sb", tag="zc_sb", bufs=1)
    c_sb = wp.tile([128, 128], F32, name=f"{R}c_sb", tag="c_sb", bufs=1)

    # ---------- P1: projection  A^T[my hslice, (p,n)] ----------
    ps_a = psB.tile([128, P * N], F32, name=f"{R}ps_a", tag="ps_big")
    for kt in range(10):
        imgs_kt = wp.tile([128, P * N], BF, name=f"{R}imgs{kt}",
                          tag="imgs_kt", bufs=2)
        nc.sync.dma_start(imgs_kt[:], imgsT[bass.ts(kt, 128), :])
        for ch in range(4):
            nc.tensor.matmul(ps_a[:, bass.ts(ch, 512)],
                             wproj_sb[:, kt, :],
                             imgs_kt[:, bass.ts(ch, 512)],
                             start=(kt == 0), stop=(kt == 9))
    # + b_proj (per-partition), cast bf16
    nc.scalar.activation(atm[:].rearrange("p n q -> p (n q)"),
                         ps_a[:], Act.Identity, bias=bproj_sb[:])
    # A batch-major copy: ab[n, p, h] via 16 PE transposes of atm[:, p, :]
    for p in range(P):
        ps_t2 = psB.tile([128, 128], BF, name=f"{R}ps_ab{p}", tag="ps_tr",
                         bufs=1)
        nc.tensor.transpose(ps_t2[:], atm[:, p, :], ident_sb[:])
        nc.scalar.copy(ab[:, p, :], ps_t2[:])
    # h0^T my slice = mean over positions (strided view [h, n, p])
    h0t_f = wp.tile([128, 128], F32, name=f"{R}h0t_f", tag="h0t_f", bufs=1)
    nc.vector.tensor_reduce(h0t_f[:], atm[:].transpose([0, 2, 1]),
                            mybir.AxisListType.X, AluOp.add)
    nc.scalar.mul(h0t_f[:], h0t_f[:], 1.0 / P)
    h0t_b = wp.tile([128, 128], BF, name=f"{R}h0t_b", tag="h0t_b", bufs=1)
    nc.scalar.copy(h0t_b[:], h0t_f[:])
    # c0 = h0 (batch-major my hidden slice)
    ps_tr0 = psB.tile([128, 128], BF, name=f"{R}ps_tr", tag="ps_tr",
                      bufs=1)
    nc.tensor.transpose(ps_tr0[:], h0t_b[:], ident_sb[:])
    nc.scalar.copy(c_sb[:], ps_tr0[:])
    c_bf = wp.tile([128, 128], BF, name=f"{R}c_bf", tag="c_bf", bufs=1)
    nc.scalar.copy(c_bf[:], ps_tr0[:])

    # scores partial [n, p] = sum_h h[n,h] * A[n,p,h]  (scale 1/32 folded
    # into the post-AG exp), packed with h^T into the per-step AllGather.
    def scores_and_ag(step, hT_ap, hb_ap):
        pay = dram.tile([PAY], U8, name=f"{R}pay{step}", tag="pay", bufs=3)
        if "noattn" not in abl:
            e_b = wp.tile([128, P, HS], BF, name=f"{R}e_{step}", tag="e_b",
                          bufs=2)
            nc.vector.tensor_mul(
                e_b[:], ab[:],
                hb_ap.unsqueeze(1).broadcast_to([128, P, HS]))
            sc_part = wp.tile([128, P], F32, name=f"{R}scp{step}",
                              tag="sc_part", bufs=2)
            nc.vector.tensor_reduce(sc_part[:], e_b[:],
                                    mybir.AxisListType.X, AluOp.add)
            nc.sync.dma_start(
                pay[HT_BYTES:PAY].bitcast(F32)
                .rearrange("(n q) -> n q", n=128),
                sc_part[:])
        nc.sync.dma_start(
            pay[0:HT_BYTES].rearrange("(p b) -> p b", p=128),
            hT_ap.bitcast(mybir.dt.uint8))
        gat = dram.tile([NCORES, PAY], U8, name=f"{R}gat{step}", tag="gat",
                        bufs=3, addr_space="Shared")
        if "noag" in abl:
            nc.sync.dma_start(gat[0], pay[:])
        else:
            nc.gpsimd.collective_compute(
                "AllGather", AluOp.bypass, replica_groups=rg,
                ins=[pay.opt()], outs=[gat.opt()])
        return gat

    gat = scores_and_ag(0, h0t_b[:], c_bf[:])

    # ---------- at_all: gather A^T from all cores (two halves, each
    # half's B-precompute overlaps the next half's AllGather) ----------
    HP = P * N // 2
    gavs = []
    for hf in range(2):
        pay_a = dram.tile([128, HP], BF, name=f"{R}pay_a{hf}",
                          tag=f"pay_a{hf}")
        nc.sync.dma_start(
            pay_a[:],
            atm[:, 8 * hf:8 * hf + 8, :].rearrange("p n q -> p (n q)"))
        gat_a = dram.tile([NCORES * 128, HP], BF, name=f"{R}gat_a{hf}",
                          tag=f"gat_a{hf}", addr_space="Shared")
        nc.gpsimd.collective_compute(
            "AllGather", AluOp.bypass, replica_groups=rg,
            ins=[pay_a.opt()], outs=[gat_a.opt()])
        gavs.append(gat_a[:].rearrange("(c p) f -> c p f", c=NCORES))

    # ---------- B precompute: b_sb[n, p, :] = A[n, :, p] @ WattnS ----------
    for hf in range(2):
        for kt in range(8):
            nc.sync.dma_start(
                at_all[:, kt, 8 * hf:8 * hf + 8, :]
                .rearrange("p n q -> p (n q)"), gavs[hf][kt])
        for p in range(8 * hf, 8 * hf + 8):
            ps_b = psB.tile([128, SC], F32, name=f"{R}ps_b{p}", tag="ps_mm",
                            bufs=3)
            for kt in range(8):
                nc.tensor.matmul(ps_b[:], at_all[:, kt, p, :],
                                 watt_sb[:, kt, :],
                                 start=(kt == 0), stop=(kt == 7))
            nc.scalar.copy(b_sb[:, p, :], ps_b[:])

    # ---------- recurrence ----------
    xv = xembT2[:].rearrange("p (k t n) -> p k t n", k=4, t=T)
    pay2 = dram.tile([N * T * 4 + T * 128 * 4], U8, name=f"{R}pay2", tag="pay2")

    def vocab_prefetch(trow):
        wyt_sb = wp.tile([128, H], BF, name=f"{R}wyt{trow}", tag="wyt_sb",
                         bufs=2)
        nc.sync.dma_start(wyt_sb[:], wyT[trow])
        return wyt_sb

    def vocab_row(trow, hT_sb, wyt_sb):
        # logsumexp partials + label logits for hs row `trow` (h_{trow+1})
        for ci, (off, ln) in enumerate(VCH):
            ps_v = psB.tile([128, 512], F32, name=f"{R}ps_v{trow}_{ci}",
                            tag="ps_mm", bufs=3)
            for kt in range(8):
                nc.tensor.matmul(ps_v[:, :ln], hT_sb[:, kt, :],
                                 wvoc_sb[:, kt, off:off + ln],
                                 start=(kt == 0), stop=False)
            nc.tensor.matmul(ps_v[:, :ln], onesrow_sb[:],
                             bvoc_sb[:, off:off + ln],
                             start=False, stop=True)
            ex_scr = wp.tile([128, 512], BF, name=f"{R}ex{trow}_{ci}",
                             tag="ex_scr", bufs=2)
            nc.scalar.activation(ex_scr[:, :ln], ps_v[:, :ln], Act.Exp,
                                 accum_out=zc_sb[:, trow, ci:ci + 1])
        # label logit (wyT host layout [T, hl, (kt, n)] matches hT_sb)
        ey_sb = wp.tile([128, H], BF, name=f"{R}ey{trow}", tag="ey_sb", bufs=2)
        nc.vector.tensor_mul(
            ey_sb[:],
            hT_sb[:].rearrange("p k n -> p (k n)"),
            wyt_sb[:])
        ps_ll = psB.tile([1, 128], F32, name=f"{R}ps_ll{trow}",
                         tag="ps_big", bufs=1)
        for ch in range(8):   # accumulate the 8 k-tiles directly in PSUM
            nc.tensor.matmul(ps_ll[:], ones1_sb[:],
                             ey_sb[:, bass.ts(ch, 128)],
                             start=(ch == 0), stop=(ch == 7))
        ll_t = wp.tile([1, 128], F32, name=f"{R}ll_t{trow}",
                       tag="ll_t", bufs=2)
        nc.scalar.copy(ll_t[:], ps_ll[:])
        nc.sync.dma_start(
            pay2[ZB:].bitcast(F32)
            .rearrange("(n t) -> t n", n=128)[trow:trow + 1, :],
            ll_t[:])

    for t in range(T):
        # one PSUM accumulates x@Wx + b + h@Wh + attn@Wattn for this step;
        # the Wx and bias matmuls run during the AllGather wait.
        xt_sb = wp.tile([128, 4, 128], BF, name=f"{R}xt{t}", tag="xt_sb",
                        bufs=3)
        nc.sync.dma_start(xt_sb[:], xv[:, :, t, :])
        if t > 0 and "novocab" not in abl:
            wyt_sb = vocab_prefetch(t - 1)
        noat = "noattn" in abl
        with tc.high_priority():
            ps_all = psB.tile([128, SC], F32, name=f"{R}ps_all{t}",
                              tag="ps_mm", bufs=3)
            for kt in range(4):
                nc.tensor.matmul(ps_all[:], xt_sb[:, kt, :], wxs_sb[:, kt, :],
                                 start=(kt == 0), stop=False)
            nc.tensor.matmul(ps_all[:], onesrow_sb[:], brow_sb[:],
                             start=False, stop=False)
            # score partials first: they head the longer post-AG branch
            if not noat:
                sc_sb = wp.tile([128, 8, P], F32, name=f"{R}sc{t}", tag="sc_sb")
                if "noag" in abl:
                    for c2 in range(8):
                        nc.sync.dma_start(
                            sc_sb[:, c2, :],
                            gat[0, HT_BYTES:PAY].bitcast(F32)
                            .rearrange("(n q) -> n q", n=128))
                else:
                    nc.sync.dma_start(
                        sc_sb[:],
                        gat[:, HT_BYTES:PAY].bitcast(F32)
                        .rearrange("c (n q) -> n c q", n=128))
                # softmax weights (no max subtraction: |scores| < ~10)
                s4 = wp.tile([128, 4, P], F32, name=f"{R}s4_{t}", tag="s4")
                nc.vector.tensor_add(s4[:], sc_sb[:, 0:4, :], sc_sb[:, 4:8, :])
                s2 = wp.tile([128, 2, P], F32, name=f"{R}s2_{t}", tag="s2")
                nc.vector.tensor_add(s2[:], s4[:, 0:2, :], s4[:, 2:4, :])
                ssum = wp.tile([128, P], F32, name=f"{R}ssum{t}", tag="ssum")
                nc.vector.tensor_add(ssum[:], s2[:, 0, :], s2[:, 1, :])
                e_w = wp.tile([128, P], F32, name=f"{R}ew{t}", tag="e_w")
                zs = wp.tile([128, 1], F32, name=f"{R}zs{t}", tag="zs")
                nc.scalar.activation(e_w[:], ssum[:], Act.Exp,
                                     scale=1.0 / 32.0, accum_out=zs[:])
                rz = wp.tile([128, 1], F32, name=f"{R}rz{t}", tag="rz")
                nc.vector.reciprocal(rz[:], zs[:])
            hT_sb = wp.tile([128, 8, 128], BF, name=f"{R}hT{t}", tag="hT_sb",
                            bufs=3)
            if "noag" in abl:
                for c2 in range(8):
                    nc.sync.dma_start(
                        hT_sb[:, c2, :],
                        gat[0, 0:HT_BYTES].bitcast(BF)
                        .rearrange("(p b) -> p b", p=128))
                for kt in range(8):
                    nc.tensor.matmul(ps_all[:], hT_sb[:, kt, :],
                                     whs_sb[:, kt, :],
                                     start=False, stop=(noat and kt == 7))
            else:
                # unpack in two halves so Wh matmuls start after the first
                gv = gat[:, 0:HT_BYTES].bitcast(BF).rearrange(
                    "c (p b) -> p c b", p=128)
                for hf in range(2):
                    nc.sync.dma_start(hT_sb[:, 4 * hf:4 * hf + 4, :],
                                      gv[:, 4 * hf:4 * hf + 4, :])
                    for kt in range(4 * hf, 4 * hf + 4):
                        nc.tensor.matmul(ps_all[:], hT_sb[:, kt, :],
                                         whs_sb[:, kt, :],
                                         start=False, stop=(noat and kt == 7))
            if not noat:
                # attention on PE: += sum_p diag(w[:,p]) @ B_p, normalized
                # w = e_w/z; diagonals built in 4 fused DVE ops so the PE
                # can start consuming after the first quarter.
                dga = wp.tile([128, P, 128], BF, name=f"{R}dga{t}", tag="dga",
                              bufs=1)
                for q in range(4):
                    nc.vector.scalar_tensor_tensor(
                        dga[:, 4 * q:4 * q + 4, :],
                        ident16[:, 4 * q:4 * q + 4, :],
                        rz[:],
                        e_w[:, 4 * q:4 * q + 4].unsqueeze(2)
                        .broadcast_to([128, 4, 128]),
                        op0=AluOp.mult, op1=AluOp.mult)
                    for p in range(4 * q, 4 * q + 4):
                        nc.tensor.matmul(ps_all[:], dga[:, p, :],
                                         b_sb[:, p, :],
                                         start=False, stop=(p == P - 1))
            # gates: [i | f | o | g] chunks of 128, read from PSUM directly
            sig = wp.tile([128, 384], BF, name=f"{R}sig{t}", tag="sig")
            tg = wp.tile([128, 128], BF, name=f"{R}tg{t}", tag="tg")
            nc.scalar.activation(sig[:], ps_all[:, 0:384], Act.Sigmoid)
            nc.scalar.activation(tg[:], ps_all[:, 384:512], Act.Tanh)
            fc = wp.tile([128, 128], F32, name=f"{R}fc{t}", tag="fc")
            ig = wp.tile([128, 128], F32, name=f"{R}ig{t}", tag="ig")
            nc.vector.tensor_mul(fc[:], sig[:, 128:256], c_sb[:])
            nc.vector.tensor_mul(ig[:], sig[:, 0:128], tg[:])
            nc.vector.tensor_add(c_sb[:], fc[:], ig[:])
            tc_t = wp.tile([128, 128], BF, name=f"{R}tc{t}", tag="tc_t")
            nc.scalar.activation(tc_t[:], c_sb[:], Act.Tanh)
            h_sl = wp.tile([128, 128], BF, name=f"{R}hsl{t}", tag="h_sl")
            nc.vector.tensor_mul(h_sl[:], sig[:, 256:384], tc_t[:])
            # transpose h slice (payload + next-step lhsT live in gat)
            ps_tr = psB.tile([128, 128], BF, name=f"{R}ps_tr{t}",
                             tag="ps_tr", bufs=1)
            nc.tensor.transpose(ps_tr[:], h_sl[:], ident_sb[:])
            hT_c = wp.tile([128, 128], BF, name=f"{R}hTc{t}", tag="hT_c",
                           bufs=2)
            nc.scalar.copy(hT_c[:], ps_tr[:])
            # scores partial for h_{t+1} + AllGather
            gat_next = scores_and_ag(t + 1, hT_c[:], h_sl[:])
        # vocab row t uses hs[t] = h_{t+1}... but row t-1 = h_t is
        # what we have gathered now (hT_sb).
        if t > 0 and "novocab" not in abl:
            vocab_row(t - 1, hT_sb, wyt_sb)
        gat = gat_next

    # tail: vocab for last row (h_T from final AG)
    if "novocab" not in abl:
        hT_last = wp.tile([128, 8, 128], BF, name=f"{R}hT_last", tag="hT_sb")
        if "noag" in abl:
            for c2 in range(8):
                nc.sync.dma_start(
                    hT_last[:, c2, :],
                    gat[0, 0:HT_BYTES].bitcast(BF)
                    .rearrange("(p b) -> p b", p=128))
        else:
            nc.sync.dma_start(
                hT_last[:],
                gat[:, 0:HT_BYTES].bitcast(BF)
                .rearrange("c (p b) -> p c b", p=128))
        vocab_row(T - 1, hT_last, vocab_prefetch(T - 1))

    # ---------- final loss ----------
    if "novocab" in abl:
        nc.vector.memset(zc_sb[:], 1.0)
    zfin = wp.tile([128, T], F32, name=f"{R}zfin", tag="zfin")
    nc.vector.tensor_reduce(zfin[:], zc_sb[:], mybir.AxisListType.X,
                            AluOp.add)
    nc.sync.dma_start(
        pay2[0:ZB].bitcast(F32).rearrange("(p b) -> p b", p=128),
        zfin[:])
    if "novocab" in abl:   # fill the label-logit half too (same size)
        nc.sync.dma_start(
            pay2[ZB:].bitcast(F32).rearrange("(p b) -> p b", p=128),
            zfin[:])
    gat2 = dram.tile([NCORES, N * T * 4 + T * 128 * 4], U8, name=f"{R}gat2",
                     tag="gat2", addr_space="Shared")
    nc.gpsimd.collective_compute(
        "AllGather", AluOp.bypass, replica_groups=rg,
        ins=[pay2.opt()], outs=[gat2.opt()])
    zg = wp.tile([128, T, 8], F32, name=f"{R}zg", tag="zg")
    lg = wp.tile([128, T, 8], F32, name=f"{R}lg", tag="lg")
    for c2 in range(8):
        nc.sync.dma_start(
            zg[:, :, c2],
            gat2[c2, 0:ZB].bitcast(F32).rearrange("(n q) -> n q", n=128))
        nc.sync.dma_start(
            lg[:, :, c2],
            gat2[c2, ZB:].bitcast(F32)
            .rearrange("(n q) -> n q", n=128))
    zred = wp.tile([128, T], F32, name=f"{R}zred", tag="zred")
    llred = wp.tile([128, T], F32, name=f"{R}llred", tag="llred")
    nc.vector.tensor_reduce(zred[:], zg[:], mybir.AxisListType.X,
                            AluOp.add)
    nc.vector.tensor_reduce(llred[:], lg[:], mybir.AxisListType.X,
                            AluOp.add)
    lse = wp.tile([128, T], F32, name=f"{R}lse", tag="lse")
    nc.scalar.activation(lse[:], zred[:], Act.Ln)
    diff = wp.tile([128, T], F32, name=f"{R}diff", tag="diff")
    nc.vector.tensor_sub(diff[:], lse[:], llred[:])
    nc.vector.tensor_mul(diff[:], diff[:], mask_sb[:])
    per_n = wp.tile([128, 1], F32, name=f"{R}per_n", tag="per_n")
    nc.vector.tensor_reduce(per_n[:], diff[:], mybir.AxisListType.X,
                            AluOp.add)
    pn_red = wp.tile([128, 1], F32, name=f"{R}pn_red", tag="pn_red")
    nc.gpsimd.partition_all_reduce(pn_red[:], per_n[:], 128,
                                   bass_isa.ReduceOp.add)
    loss_sb = wp.tile([1, 1], F32, name=f"{R}loss_sb", tag="loss_sb")
    nc.scalar.mul(loss_sb[:], pn_red[0:1, :], 1.0 / N)
    nc.sync.dma_start(loss_out[:], loss_sb[:])


def host_prep(inputs, T):
    """Build the 8 per-core input maps (all numpy)."""
    g = {k: np.asarray(v) for k, v in inputs.items()}
    images, captions = g["images"], g["captions"]
    W_embed, W_proj, b_proj = g["W_embed"], g["W_proj"], g["b_proj"]
    Wx, Wh, Wattn, b = g["Wx"], g["Wh"], g["Wattn"], g["b"]
    W_vocab, b_vocab = g["W_vocab"], g["b_vocab"]

    cap = np.asarray(captions)
    cap_in = cap[:, :T]
    cap_out = cap[:, 1:T + 1]
    x_emb = W_embed[cap_in]                      # [N, T, WD]
    xembT2 = np.ascontiguousarray(               # [128, (kt, t, n)]
        x_emb.transpose(2, 1, 0).reshape(4, 128, T, N)
        .transpose(1, 0, 2, 3).reshape(128, 4 * T * N)).astype(BF16)
    imgsT = np.ascontiguousarray(                # (cin, p, n)
        images.reshape(N, CIN, P).transpose(1, 2, 0).reshape(CIN, P * N)
    ).astype(BF16)
    mask = (cap_out != 0).astype(np.float32)     # [N, T]
    ident = np.eye(128, dtype=BF16)
    ones1 = np.ones((128, 1), dtype=BF16)
    onesrow = np.ones((1, 128), dtype=BF16)

    # label weight vectors, [H, N, T] -> per t: [hl, (kt, n)]
    wy = W_vocab[:, cap_out]                     # [H, N, T]
    wy_t = wy.reshape(8, 128, N, T).transpose(3, 1, 0, 2)  # [T, hl, kt, n]

    in_maps = []
    for c in range(NCORES):
        hsl = slice(128 * c, 128 * (c + 1))
        idx = np.concatenate([g4 * H + 128 * c + np.arange(128)
                              for g4 in range(4)])
        vsl = slice(VS * c, VS * (c + 1))
        wyc = wy_t.copy()
        nm = np.zeros(N, dtype=wy_t.dtype)
        nm[16 * c:16 * (c + 1)] = 1
        wyc *= nm[None, None, None, :]
        in_maps.append({
            "imgsT": imgsT,
            "wprojS": np.ascontiguousarray(W_proj[:, hsl]).astype(BF16),
            "bprojS": np.ascontiguousarray(b_proj[hsl, None]).astype(np.float32),
            "xembT2": xembT2,
            "WxS": np.ascontiguousarray(Wx[:, idx]).astype(BF16),
            "brow": np.ascontiguousarray(b[None, idx]).astype(BF16),
            "WhS": np.ascontiguousarray(Wh[:, idx]).astype(BF16),
            "WattnS": np.ascontiguousarray(Wattn[:, idx]).astype(BF16),
            "wvoc": np.ascontiguousarray(W_vocab[:, vsl]).astype(BF16),
            "bvoc": np.ascontiguousarray(b_vocab[None, vsl]).astype(BF16),
            "wyT": np.ascontiguousarray(
                wyc.reshape(T, 128, H)).astype(BF16),
            "mask_f": mask,
            "ident": ident,
            "ones1": ones1,
            "onesrow": onesrow,
        })
    host_by = float(np.sum(mask.astype(np.float64) *
                           np.asarray(b_vocab, np.float64)[cap_out]) / N)
    return in_maps, host_by


_CACHE = {}


def _get_built(T, reps=1):
    if (T, reps) not in _CACHE:
        _CACHE[(T, reps)] = build(T, reps)
    return _CACHE[(T, reps)]


def run(inputs, T=30):
    nc = _get_built(T)
    in_maps, host_by = host_prep(inputs, T)
    res = run_bass_kernel_spmd(nc, in_maps, core_ids=list(range(NCORES)))
    dev_loss = float(res.results[0]["loss"][0, 0])
    return np.float32(dev_loss - host_by)


def kernel(**inputs) -> np.ndarray:
    return run(inputs, T=30)
